# revision 40
# baseline (speedup 1.0000x reference)
"""Causal self-attention (B=8, T=1024, C=768, NH=12) on 8 TRN2 NeuronCores.

Sharding: pure data parallel - one batch element per core, no collectives.

Host side: x is pre-transposed to [C, T] and shipped twice - as bf16 (for
the v projection) and as fp8e4m3 (for the qk projection); w_attn/w_proj are
cast to bf16 and the q/k half of w_attn additionally to fp8e4m3 (w8).
Biases stay fp32.

Per-core kernel (Bass/Tile):
  - The fp8 qk inputs (w8 + x8, 1.9 MB) load first so the exp stream starts
    ~8.5us in; bf16 x and wv stream in behind them for the v projection.
  - qk projection in fp8e4m3 with MatmulPerfMode.DoubleRow: 3 accumulation
    steps of K=256 ([128, 2, *] operand pairs), 4x fewer PE cycles than the
    bf16 equivalent. Bias-add + PSUM->SBUF move on DVE (per-partition bias
    battn_pm). v projection and output projection stay bf16 (fp8 there fails
    the 2e-2 accuracy budget; qk errors are damped through exp).
  - Attention per (head pair, 512-wide tq block) "unit": ST = kT.T @ qT ->
    PSUM [tk=128, tq] (2 heads in PE row groups 0-63/64-127), exp on ACT
    into SBUF `ut` (the engine-time floor: ~60us of exp), diagonal-block
    causal mask via gpsimd affine_select on the idle Pool engine.
  - PV is oriented [tq, hd]: per 128-wide tq chunk j and head,
    y[tq, 65] += U[tk, tq-chunk].T @ v_aug[tk, head] accumulated over tk as
    one PSUM group per (j, head) (PSUM pending-zero is bank-granular, so
    groups in a bank must not interleave). Column 64 (ones in v_aug) yields
    the softmax row-sums, so normalization is per-partition: reciprocal +
    one broadcast multiply per psum tile on DVE - no PE broadcast matmuls.
  - y[tq, (h,hd)] tiles are transposed back to yT [c-chunk, tq] for the
    output projection via SBUF->SBUF DMA XBAR transposes (the DMA path is
    idle mid-stream); only the last two units - whose yT sits on the tail
    critical path - use the lower-latency PE-transpose + DVE-copy path.
    Output projection: stationary yT, moving w_proj, bias on DVE, DMA out.
  - PSUM (8 banks): "big" = ST psum ring (2 slots x 2 banks, nothing else -
    sharing it halves the ST/exp double-buffering), "aux" (1 slot x 2 banks)
    for qk/v/out projection psums, "pvy" (2 slots x 1 bank) for the PV
    accumulators and the y-transpose psum.
  - Schedule: emission order defines both scheduler priority AND the
    dataflow (a consumer emitted before its producer is a race, not a
    reorderable dependency - CoreSim's initialized-memory check catches
    this). Attention units alternate b=0 (exp-light) and b=1 (exp-heavy)
    blocks so the ACT exp stream has no holes, and PV work is woven with a
    LAG OF TWO units (PV of unit u between unit u+2's ST tiles, draining
    double at the end) so the early ST blocks are not held back by the
    DMA-gated v-tile backlog. qk/v projection halves and out-proj tiles
    ride as fillers in the remaining ST slots.

TimelineSim: 101.5us vs 168.3us for the previous kernel (1.66x). HW-verified
rel err 9.7e-3 (fp8 qk path dominates; bf16-only baseline was 3.1e-3).
"""

import numpy as np
import ml_dtypes

import concourse.bass as bass
import concourse.bacc as bacc
import concourse.tile as tile
from concourse import mybir
from concourse.bass_utils import run_bass_kernel_spmd

B, T, C = 8, 1024, 768
NH, HD = 12, 64
P = 128
KC = C // P          # 6 k-tiles over C
KT = T // P          # 8 tiles over T
NQK = 2 * C // P     # 12 m-tiles for q+k
NHP = NH // 2        # 6 head pairs
TQB = 512            # tq block (one PSUM bank of fp32)
NB = T // TQB        # 2 tq blocks
NJ = TQB // P        # 4 tq chunks per block
VW = HD + 1          # 65: v columns + ones column per head

F32 = mybir.dt.float32
BF16 = mybir.dt.bfloat16
FP8 = mybir.dt.float8e4
FT = mybir.ActivationFunctionType
ALU = mybir.AluOpType


def build_program():
    nc = bacc.Bacc("TRN2", target_bir_lowering=False, debug=False)
    xb_d = nc.dram_tensor("xbT", [C, T], BF16, kind="ExternalInput").ap()
    wab_d = nc.dram_tensor("wab", [C, 3 * C], BF16, kind="ExternalInput").ap()
    w8_d = nc.dram_tensor("w8", [C, 2 * C], FP8, kind="ExternalInput").ap()
    ba_d = nc.dram_tensor("b_attn", [3 * C], F32, kind="ExternalInput").ap()
    wpb_d = nc.dram_tensor("wpb", [C, C], BF16, kind="ExternalInput").ap()
    bp_d = nc.dram_tensor("b_proj", [C], F32, kind="ExternalInput").ap()
    out_d = nc.dram_tensor("out", [T, C], F32, kind="ExternalOutput").ap()

    from contextlib import ExitStack

    with tile.TileContext(nc) as tc:
        with ExitStack() as ctx:
            _body(ctx, tc, xb_d, wab_d, w8_d, ba_d, wpb_d, bp_d, out_d)
    nc.compile()
    return nc


def _body(ctx, tc, xb_d, wab_d, w8_d, ba_d, wpb_d, bp_d, out_d):
    nc = tc.nc

    const = ctx.enter_context(tc.tile_pool(name="const", bufs=1))
    persist = ctx.enter_context(tc.tile_pool(name="persist", bufs=1))
    upool = ctx.enter_context(tc.tile_pool(name="upool", bufs=24))
    ypool = ctx.enter_context(tc.tile_pool(name="ypool", bufs=2))
    rcpool = ctx.enter_context(tc.tile_pool(name="rcpool", bufs=2))
    otpool = ctx.enter_context(tc.tile_pool(name="otpool", bufs=4))

    # constants ------------------------------------------------------------
    ident = const.tile([P, P], BF16)
    nc.gpsimd.memset(ident, 0.0)
    nc.gpsimd.affine_select(
        out=ident, in_=ident, compare_op=ALU.not_equal,
        fill=1.0, base=0, pattern=[[-1, P]], channel_multiplier=1,
    )
    # b_attn for q/k as per-partition scalars: [p, m] with b[128m + p]
    battn_pm = const.tile([P, NQK], F32)
    # biases broadcast along partitions on-chip (a stride-0 partition DMA is
    # modeled/executed as a full 128x transfer): DMA one row, gpsimd bcast
    brow = const.tile([P, 2, C], F32)
    bv_b = const.tile([P, C], F32)
    bp_b = const.tile([P, C], F32)

    # persistent SBUF tensors ---------------------------------------------
    xT = persist.tile([P, KC, T], BF16)          # [128, 6, 1024] 12 KB/par
    xT8 = persist.tile([P, KC // 2, 2, T], FP8)  # fp8 copy for qk DoubleRow
    w8_sb = persist.tile([P, KC // 2, 2, 2 * C], FP8)  # [128, 3, 2, 1536]
    wv_sb = persist.tile([P, KC, C], BF16)       # 9 KB
    wp_sb = persist.tile([P, KC, C], BF16)       # 9 KB
    qkT = persist.tile([P, NQK, T], BF16)        # [128, 12, 1024] 24 KB
    vaug = persist.tile([P, KT, NH * VW], BF16)  # [128, 8, 780] 12.2 KB
    yT = persist.tile([P, NHP, T], BF16)         # [128, 6, 1024] 12 KB

    # input DMAs in priority order; x arrives host-transposed [C, T] so xT
    # chunks are plain contiguous loads; the qk weight half loads as 6
    # contiguous row-chunk DMAs (all 12 m-tiles at once)
    for k in range(KC):
        if k % 2 == 0:
            t = k // 2
            nc.sync.dma_start(
                out=w8_sb[:, t, :, :],
                in_=w8_d[t * 2 * P : (t + 1) * 2 * P, :].rearrange(
                    "(i p) m -> p i m", p=P
                ),
            )
        nc.sync.dma_start(
            out=xT[:, k, :], in_=xb_d[k * P : (k + 1) * P, :]
        )
        # fp8 copy for the qk projection (channel order is unchanged:
        # free index (t, i) == chunk 2t+i); alternate Pool/DVE
        eng = nc.gpsimd if k % 2 == 0 else nc.vector
        eng.tensor_copy(out=xT8[:, k // 2, k % 2, :], in_=xT[:, k, :])
    nc.sync.dma_start(out=brow[0:1, 0, :], in_=ba_d[None, 2 * C : 3 * C])
    nc.sync.dma_start(out=brow[0:1, 1, :], in_=bp_d[None, :])
    nc.sync.dma_start(
        out=battn_pm, in_=ba_d[0 : 2 * C].rearrange("(m p) -> p m", p=P)
    )
    for k in range(KC):
        nc.sync.dma_start(
            out=wv_sb[:, k, :], in_=wab_d[k * P : (k + 1) * P, 2 * C : 3 * C]
        )
    for k in range(KC):
        nc.sync.dma_start(out=wp_sb[:, k, :], in_=wpb_d[k * P : (k + 1) * P, :])

    # ones columns in vaug (per head col 64), then bias broadcasts (Pool)
    vhe = vaug[:, :, :].rearrange("p t (h e) -> p t h e", e=VW)
    nc.gpsimd.memset(vhe[:, :, :, HD : HD + 1], 1.0)
    nc.gpsimd.partition_broadcast(out_ap=bv_b, in_ap=brow[0:1, 0, :])
    nc.gpsimd.partition_broadcast(out_ap=bp_b, in_ap=brow[0:1, 1, :])

    # PSUM pools -----------------------------------------------------------
    pbig = ctx.enter_context(tc.tile_pool(name="pbig", bufs=2, space="PSUM"))
    paux = ctx.enter_context(tc.tile_pool(name="paux", bufs=1, space="PSUM"))
    ppvy = ctx.enter_context(tc.tile_pool(name="ppvy", bufs=2, space="PSUM"))

    # ---- building blocks -------------------------------------------------
    def qk_tile(m, n):
        ps = paux.tile([P, TQB], F32, name=f"qkps{m}_{n}", tag="aux")
        for t in range(KC // 2):
            nc.tensor.matmul(
                ps[:, :],
                w8_sb[:, t, :, m * P : (m + 1) * P],
                xT8[:, t, :, n * TQB : (n + 1) * TQB],
                start=(t == 0),
                stop=(t == KC // 2 - 1),
                perf_mode=mybir.MatmulPerfMode.DoubleRow,
            )
        # bias-add + PSUM->SBUF move on DVE (ACT is the exp bottleneck)
        nc.vector.tensor_tensor(
            out=qkT[:, m, n * TQB : (n + 1) * TQB],
            in0=ps[:, :],
            in1=battn_pm[:, m : m + 1].to_broadcast([P, TQB]),
            op=ALU.add,
        )

    def v_tile(tt, n):
        for _, cl in v_items(tt, n):
            cl()

    def v_items(tt, n):
        """v projection half-tile as 3 small work items (2 k-chunks each)."""
        nsz = min(TQB, C - n * TQB)  # 512, 256
        state = {}

        def chunk(j):
            if "ps" not in state:
                state["ps"] = paux.tile(
                    [P, TQB], F32, name=f"vps{tt}_{n}", tag="aux"
                )
            ps = state["ps"]
            for k in (2 * j, 2 * j + 1):
                nc.tensor.matmul(
                    ps[:, :nsz],
                    xT[:, k, tt * P : (tt + 1) * P],
                    wv_sb[:, k, n * TQB : n * TQB + nsz],
                    start=(k == 0),
                    stop=(k == KC - 1),
                )
            if j == 2:
                nh0 = n * TQB // HD
                nh = nsz // HD
                nc.vector.tensor_tensor(
                    out=vhe[:, tt, nh0 : nh0 + nh, 0:HD],
                    in0=ps[:, :nsz].rearrange("p (h e) -> p h e", e=HD),
                    in1=bv_b[:, n * TQB : n * TQB + nsz].rearrange(
                        "p (h e) -> p h e", e=HD
                    ),
                    op=ALU.add,
                )

        est = (430, 215)[n]
        return [(est, lambda j=j: chunk(j)) for j in range(3)]

    def out_tile(m, pool=None, tag="aux"):
        for _, cl in out_items(m, pool, tag):
            cl()

    def out_items(m, pool=None, tag="aux"):
        """out-projection tile as 6 small work items (2 k-chunks per n)."""
        state = {}

        def chunk(n, j):
            if "ps" not in state:
                state["ps"] = (pool or paux).tile(
                    [P, NB, TQB], F32, name=f"ops{m}", tag=tag
                )
                state["ot"] = otpool.tile([P, C], F32, name=f"ot{m}", tag="ot")
            ps, ot = state["ps"], state["ot"]
            nsz = min(TQB, C - n * TQB)
            for k in (2 * j, 2 * j + 1):
                nc.tensor.matmul(
                    ps[:, n, :nsz],
                    yT[:, k, m * P : (m + 1) * P],
                    wp_sb[:, k, n * TQB : n * TQB + nsz],
                    start=(k == 0),
                    stop=(k == KC - 1),
                )
            if j == 2:
                nc.vector.tensor_tensor(
                    out=ot[:, n * TQB : n * TQB + nsz],
                    in0=ps[:, n, :nsz],
                    in1=bp_b[:, n * TQB : n * TQB + nsz],
                    op=ALU.add,
                )
                nc.sync.dma_start(
                    out=out_d.rearrange("(t p) c -> p t c", p=P)[
                        :, m : m + 1, n * TQB : n * TQB + nsz
                    ],
                    in_=ot[:, None, n * TQB : n * TQB + nsz],
                )

        return [
            ((430, 215)[n], lambda n=n, j=j: chunk(n, j))
            for n in range(NB)
            for j in range(3)
        ]

    def attn_ST(hp, b, workq):
        """ST + exp + mask for one head pair and tq block; returns the ut
        tiles for attn_PV. Between ST tiles, drains work items from `workq`
        up to the PE slack under each exp's latency."""
        ntk = 4 * (b + 1)
        uts = []
        for tk in range(ntk):
            off = max(0, tk * P - b * TQB)
            nn = TQB - off
            diag = tk * P >= b * TQB
            pst = pbig.tile([P, 2, TQB], F32, name="pst", tag="big")
            for h in range(2):
                lo, hi = 64 * h, 64 * h + 64
                nc.tensor.matmul(
                    pst[:, h, off:TQB],
                    qkT[lo:hi, 6 + hp, tk * P : (tk + 1) * P],
                    qkT[lo:hi, hp, b * TQB + off : (b + 1) * TQB],
                    start=True,
                    stop=True,
                )
            ut = upool.tile([P, 2, TQB], BF16, name="ut")
            nc.scalar.activation(
                out=ut[:, :, off:TQB],
                in_=pst[:, :, off:TQB],
                func=FT.Exp,
                scale=0.125,
            )
            if diag:
                # zero the strictly-upper triangle of the diagonal block
                # (keep where tq_col - tk_row >= 0), on the Pool engine
                nc.gpsimd.affine_select(
                    out=ut[:, :, off : off + P],
                    in_=ut[:, :, off : off + P],
                    compare_op=ALU.is_ge,
                    fill=0.0,
                    base=0,
                    pattern=[[0, 2], [1, P]],
                    channel_multiplier=-1,
                )
            uts.append(ut)
            if workq:
                workq.pop(0)()
        for cl in workq:
            cl()
        del workq[:]
        return uts

    def attn_PV_items(hp, b, uts, dma_ytr=False):
        """PV accumulation + normalization for one unit as a list of small
        closures, so they can be woven between the next unit's ST tiles."""
        state = {}

        def group(j):
            if "pvy" not in state:
                state["pvy"] = [
                    ppvy.tile(
                        [P, 2, 2, VW], F32, name=f"pvy{t}_{hp}_{b}", tag="pvy"
                    )
                    for t in range(2)
                ]
            pvy = state["pvy"]
            # one PSUM accumulation group per (j, h): all tk's consecutively
            # (PSUM pending-zero is bank-granular, groups in a bank must not
            # interleave)
            last = 4 * b + j
            for h in range(2):
                for tk in range(last + 1):
                    nc.tensor.matmul(
                        pvy[j // 2][:, j % 2, h, 0:VW],
                        uts[tk][:, h, j * P : (j + 1) * P],
                        vaug[:, tk, (2 * hp + h) * VW : (2 * hp + h + 1) * VW],
                        start=(tk == 0),
                        stop=(tk == last),
                    )

        def norm():
            pvy = state["pvy"]
            # per-partition reciprocal of the row-sum column, then one
            # broadcast multiply per psum tile
            rc = rcpool.tile([P, NJ, 2, 1], F32, name="rc")
            ysb = ypool.tile([P, NJ, 2, HD], BF16, name="ysb")
            with tc.high_priority():
                with nc.allow_low_precision(reason="softmax normalization"):
                    for t in range(2):
                        nc.vector.reciprocal(
                            out=rc[:, 2 * t : 2 * t + 2, :, :],
                            in_=pvy[t][:, :, :, HD : HD + 1],
                        )
                for t in range(2):
                    nc.vector.tensor_tensor(
                        out=ysb[:, 2 * t : 2 * t + 2, :, :],
                        in0=pvy[t][:, :, :, 0:HD],
                        in1=rc[:, 2 * t : 2 * t + 2, :, :].to_broadcast(
                            [P, 2, 2, HD]
                        ),
                        op=ALU.mult,
                    )
                # transpose y [tq, (h hd)] -> yT [(h hd), tq] per tq chunk
                if dma_ytr:
                    # off the PE/DVE path: SBUF->SBUF XBAR transpose (the
                    # DMA path is idle mid-stream; not used for the last
                    # units whose yT is on the tail critical path)
                    for j in range(NJ):
                        nc.sync.dma_start_transpose(
                            out=yT[
                                :, hp, b * TQB + j * P : b * TQB + (j + 1) * P
                            ],
                            in_=ysb[:, j, :, :],
                        )
                else:
                    ytr = ppvy.tile([P, NJ, P], BF16, name="ytr", tag="pvy")
                    for j in range(NJ):
                        nc.tensor.transpose(
                            ytr[:, j, :], ysb[:, j, :, :], ident[:]
                        )
                    nc.vector.tensor_copy(
                        out=yT[:, hp, b * TQB : (b + 1) * TQB].rearrange(
                            "p (j f) -> p j f", j=NJ
                        ),
                        in_=ytr[:, :, :],
                    )

        return [
            ((4 * b + j + 1) * 2 * 27 + 30, lambda j=j: group(j))
            for j in range(NJ)
        ] + [(350, norm)]

    # ---- emission schedule ----------------------------------------------
    # One list of (hp, b) attention units in processing order; ST blocks are
    # software-pipelined one unit ahead of PV blocks so the PE stream never
    # waits on the exp (ACT) chain. Projection halves and out-proj tiles
    # ride as PE filler inside the ST tk loops.
    qk_tile(0, 0)
    qk_tile(6, 0)
    v_tile(0, 0)
    v_tile(1, 0)
    v_tile(2, 0)
    v_tile(3, 0)

    # unit order front-loads the exp-heavy b=1 blocks so the ACT exp stream
    # has no holes; (5,0) stays ahead of the last two b=1 units so the b=0
    # output tiles get a head start before the tail
    UNITS = [
        (0, 0), (1, 0), (0, 1), (1, 1), (2, 0), (2, 1),
        (3, 0), (3, 1), (4, 0), (5, 0), (4, 1), (5, 1),
    ]
    FILL = {
        (0, 0): [(qk_tile, 1, 0), (qk_tile, 7, 0), (qk_tile, 0, 1),
                 (qk_tile, 6, 1)],
        (1, 0): [(qk_tile, 1, 1), (qk_tile, 7, 1), (v_tile, 0, 0),
                 (v_tile, 1, 0), (v_tile, 2, 0), (v_tile, 3, 0)],
        (0, 1): [(qk_tile, 2, 0), (qk_tile, 8, 0), (v_tile, 4, 0),
                 (v_tile, 5, 0)],
        (1, 1): [(qk_tile, 2, 1), (qk_tile, 8, 1), (v_tile, 6, 0),
                 (v_tile, 7, 0)],
        (2, 0): [(qk_tile, 3, 0), (qk_tile, 9, 0), (v_tile, 0, 1)],
        (2, 1): [(qk_tile, 3, 1), (qk_tile, 9, 1), (v_tile, 1, 1)],
        (3, 0): [(qk_tile, 4, 0), (qk_tile, 10, 0), (v_tile, 2, 1)],
        (3, 1): [(qk_tile, 5, 0), (qk_tile, 11, 0), (v_tile, 3, 1)],
        (4, 0): [(qk_tile, 4, 1), (qk_tile, 10, 1), (v_tile, 4, 1)],
        (5, 0): [(qk_tile, 5, 1), (qk_tile, 11, 1), (v_tile, 5, 1)],
        (4, 1): [(v_tile, 6, 1), (v_tile, 7, 1)],
        (5, 1): [(out_tile, 0), (out_tile, 1), (out_tile, 2),
                 (out_tile, 3)],
    }

    def _weave(a, b_):
        out = []
        while a or b_:
            if a:
                out.append(a.pop(0))
            if b_:
                out.append(b_.pop(0))
        return out

    qk_tile(0, 0)
    qk_tile(6, 0)
    # lag-2 PV weaving: unit u's PV work is woven during unit u+2, so the
    # early ST blocks (and their exps) are not held back by the v-tile
    # backlog; the last two units drain double so the tail stays one PV deep
    from collections import deque

    pend = deque()
    for ui, unit in enumerate(UNITS):
        hp, b = unit
        drain = []
        while pend and (len(pend) >= 2 or ui >= len(UNITS) - 2):
            drain.extend(cl for _, cl in pend.popleft())
        fillers = [(lambda f=u[0], args=u[1:]: f(*args)) for u in FILL[unit]]
        work = _weave(drain, fillers)
        uts = attn_ST(hp, b, work)
        pend.append(attn_PV_items(hp, b, uts, dma_ytr=ui < len(UNITS) - 2))
    for lst in pend:
        for _, cl in lst:
            cl()
    for m in range(4, KT):
        if m % 2 == 1:
            out_tile(m, pool=pbig, tag="big")
        else:
            out_tile(m)

_prog_cache = {}


def _get_program():
    if "nc" not in _prog_cache:
        _prog_cache["nc"] = build_program()
    return _prog_cache["nc"]


def kernel(x, w_attn, b_attn, w_proj, b_proj, _trace=False):
    nc = _get_program()
    bf = ml_dtypes.bfloat16
    xb = np.ascontiguousarray(
        np.asarray(x, dtype=np.float32).astype(bf).transpose(0, 2, 1)
    )
    wab = np.ascontiguousarray(np.asarray(w_attn, dtype=np.float32).astype(bf))
    w8 = np.ascontiguousarray(
        np.asarray(w_attn[:, : 2 * C], dtype=np.float32).astype(
            ml_dtypes.float8_e4m3
        )
    )
    wpb = np.ascontiguousarray(np.asarray(w_proj, dtype=np.float32).astype(bf))
    b_attn = np.ascontiguousarray(np.asarray(b_attn, dtype=np.float32))
    b_proj = np.ascontiguousarray(np.asarray(b_proj, dtype=np.float32))
    in_maps = [
        {
            "xbT": xb[b],
            "wab": wab,
            "w8": w8,
            "b_attn": b_attn,
            "wpb": wpb,
            "b_proj": b_proj,
        }
        for b in range(B)
    ]
    res = run_bass_kernel_spmd(nc, in_maps, list(range(B)), trace=_trace)
    out = np.stack([res.results[i]["out"] for i in range(B)], axis=0)
    if _trace:
        kernel.last_results = res
    return out


# revision 42
# speedup vs baseline: 1.0010x; 1.0010x over previous
"""Causal self-attention (B=8, T=1024, C=768, NH=12) on 8 TRN2 NeuronCores.

Sharding: pure data parallel - one batch element per core, no collectives.

Host side: x is pre-transposed to [C, T] and shipped twice - as bf16 (for
the v projection) and as fp8e4m3 (for the qk projection); w_attn/w_proj are
cast to bf16 and the q/k half of w_attn additionally to fp8e4m3 (w8).
Biases stay fp32.

Per-core kernel (Bass/Tile):
  - The fp8 qk inputs (w8 + x8, 1.9 MB) load first so the exp stream starts
    ~8.5us in; bf16 x and wv stream in behind them for the v projection.
  - qk projection in fp8e4m3 with MatmulPerfMode.DoubleRow: 3 accumulation
    steps of K=256 ([128, 2, *] operand pairs), 4x fewer PE cycles than the
    bf16 equivalent. Bias-add + PSUM->SBUF move on DVE (per-partition bias
    battn_pm). v projection and output projection stay bf16 (fp8 there fails
    the 2e-2 accuracy budget; qk errors are damped through exp).
  - Attention per (head pair, 512-wide tq block) "unit": ST = kT.T @ qT ->
    PSUM [tk=128, tq] (2 heads in PE row groups 0-63/64-127), exp on ACT
    into SBUF `ut` (the engine-time floor: ~60us of exp), diagonal-block
    causal mask via gpsimd affine_select on the idle Pool engine.
  - PV is oriented [tq, hd]: per 128-wide tq chunk j and head,
    y[tq, 65] += U[tk, tq-chunk].T @ v_aug[tk, head] accumulated over tk as
    one PSUM group per (j, head) (PSUM pending-zero is bank-granular, so
    groups in a bank must not interleave). Column 64 (ones in v_aug) yields
    the softmax row-sums, so normalization is per-partition: reciprocal +
    one broadcast multiply per psum tile on DVE - no PE broadcast matmuls.
  - y[tq, (h,hd)] tiles are transposed back to yT [c-chunk, tq] for the
    output projection via SBUF->SBUF DMA XBAR transposes (the DMA path is
    idle mid-stream); only the last two units - whose yT sits on the tail
    critical path - use the lower-latency PE-transpose + DVE-copy path.
    Output projection: stationary yT, moving w_proj, bias on DVE, DMA out.
  - PSUM (8 banks): "big" = ST psum ring (2 slots x 2 banks, nothing else -
    sharing it halves the ST/exp double-buffering), "aux" (1 slot x 2 banks)
    for qk/v/out projection psums, "pvy" (2 slots x 1 bank) for the PV
    accumulators and the y-transpose psum.
  - Schedule: emission order defines both scheduler priority AND the
    dataflow (a consumer emitted before its producer is a race, not a
    reorderable dependency - CoreSim's initialized-memory check catches
    this). Attention units alternate b=0 (exp-light) and b=1 (exp-heavy)
    blocks so the ACT exp stream has no holes, and PV work is woven with a
    LAG OF TWO units (PV of unit u between unit u+2's ST tiles, draining
    double at the end) so the early ST blocks are not held back by the
    DMA-gated v-tile backlog. qk/v projection halves and out-proj tiles
    ride as fillers in the remaining ST slots.

TimelineSim: 101.5us vs 168.3us for the previous kernel (1.66x). HW-verified
rel err 9.7e-3 (fp8 qk path dominates; bf16-only baseline was 3.1e-3).
"""

import numpy as np
import ml_dtypes

import concourse.bass as bass
import concourse.bacc as bacc
import concourse.tile as tile
from concourse import mybir
from concourse.bass_utils import run_bass_kernel_spmd

B, T, C = 8, 1024, 768
NH, HD = 12, 64
P = 128
KC = C // P          # 6 k-tiles over C
KT = T // P          # 8 tiles over T
NQK = 2 * C // P     # 12 m-tiles for q+k
NHP = NH // 2        # 6 head pairs
TQB = 512            # tq block (one PSUM bank of fp32)
NB = T // TQB        # 2 tq blocks
NJ = TQB // P        # 4 tq chunks per block
VW = HD + 1          # 65: v columns + ones column per head

F32 = mybir.dt.float32
BF16 = mybir.dt.bfloat16
FP8 = mybir.dt.float8e4
FT = mybir.ActivationFunctionType
ALU = mybir.AluOpType


def build_program():
    nc = bacc.Bacc("TRN2", target_bir_lowering=False, debug=False)
    xb_d = nc.dram_tensor("xbT", [C, T], BF16, kind="ExternalInput").ap()
    wab_d = nc.dram_tensor("wab", [C, 3 * C], BF16, kind="ExternalInput").ap()
    w8_d = nc.dram_tensor("w8", [C, 2 * C], FP8, kind="ExternalInput").ap()
    ba_d = nc.dram_tensor("b_attn", [3 * C], F32, kind="ExternalInput").ap()
    wpb_d = nc.dram_tensor("wpb", [C, C], BF16, kind="ExternalInput").ap()
    bp_d = nc.dram_tensor("b_proj", [C], F32, kind="ExternalInput").ap()
    out_d = nc.dram_tensor("out", [T, C], F32, kind="ExternalOutput").ap()

    from contextlib import ExitStack

    with tile.TileContext(nc) as tc:
        with ExitStack() as ctx:
            _body(ctx, tc, xb_d, wab_d, w8_d, ba_d, wpb_d, bp_d, out_d)
    nc.compile()
    return nc


def _body(ctx, tc, xb_d, wab_d, w8_d, ba_d, wpb_d, bp_d, out_d):
    nc = tc.nc

    const = ctx.enter_context(tc.tile_pool(name="const", bufs=1))
    persist = ctx.enter_context(tc.tile_pool(name="persist", bufs=1))
    upool = ctx.enter_context(tc.tile_pool(name="upool", bufs=24))
    ypool = ctx.enter_context(tc.tile_pool(name="ypool", bufs=2))
    rcpool = ctx.enter_context(tc.tile_pool(name="rcpool", bufs=2))
    otpool = ctx.enter_context(tc.tile_pool(name="otpool", bufs=4))

    # constants ------------------------------------------------------------
    ident = const.tile([P, P], BF16)
    nc.gpsimd.memset(ident, 0.0)
    nc.gpsimd.affine_select(
        out=ident, in_=ident, compare_op=ALU.not_equal,
        fill=1.0, base=0, pattern=[[-1, P]], channel_multiplier=1,
    )
    # b_attn for q/k as per-partition scalars: [p, m] with b[128m + p]
    battn_pm = const.tile([P, NQK], F32)
    # biases broadcast along partitions on-chip (a stride-0 partition DMA is
    # modeled/executed as a full 128x transfer): DMA one row, gpsimd bcast
    brow = const.tile([P, 2, C], F32)
    bv_b = const.tile([P, C], F32)
    bp_b = const.tile([P, C], F32)

    # persistent SBUF tensors ---------------------------------------------
    xT = persist.tile([P, KC, T], BF16)          # [128, 6, 1024] 12 KB/par
    xT8 = persist.tile([P, KC // 2, 2, T], FP8)  # fp8 copy for qk DoubleRow
    w8_sb = persist.tile([P, KC // 2, 2, 2 * C], FP8)  # [128, 3, 2, 1536]
    wv_sb = persist.tile([P, KC, C], BF16)       # 9 KB
    wp_sb = persist.tile([P, KC, C], BF16)       # 9 KB
    qkT = persist.tile([P, NQK, T], BF16)        # [128, 12, 1024] 24 KB
    vaug = persist.tile([P, KT, NH * VW], BF16)  # [128, 8, 780] 12.2 KB
    yT = persist.tile([P, NHP, T], BF16)         # [128, 6, 1024] 12 KB

    # input DMAs in priority order; x arrives host-transposed [C, T] so xT
    # chunks are plain contiguous loads; the qk weight half loads as 6
    # contiguous row-chunk DMAs (all 12 m-tiles at once)
    for k in range(KC):
        if k % 2 == 0:
            t = k // 2
            nc.sync.dma_start(
                out=w8_sb[:, t, :, :],
                in_=w8_d[t * 2 * P : (t + 1) * 2 * P, :].rearrange(
                    "(i p) m -> p i m", p=P
                ),
            )
        nc.sync.dma_start(
            out=xT[:, k, :], in_=xb_d[k * P : (k + 1) * P, :]
        )
        # fp8 copy for the qk projection (channel order is unchanged:
        # free index (t, i) == chunk 2t+i); alternate Pool/DVE
        eng = nc.gpsimd if k % 2 == 0 else nc.vector
        eng.tensor_copy(out=xT8[:, k // 2, k % 2, :], in_=xT[:, k, :])
    nc.sync.dma_start(out=brow[0:1, 0, :], in_=ba_d[None, 2 * C : 3 * C])
    nc.sync.dma_start(out=brow[0:1, 1, :], in_=bp_d[None, :])
    nc.sync.dma_start(
        out=battn_pm, in_=ba_d[0 : 2 * C].rearrange("(m p) -> p m", p=P)
    )
    for k in range(KC):
        nc.sync.dma_start(
            out=wv_sb[:, k, :], in_=wab_d[k * P : (k + 1) * P, 2 * C : 3 * C]
        )
    for k in range(KC):
        nc.sync.dma_start(out=wp_sb[:, k, :], in_=wpb_d[k * P : (k + 1) * P, :])

    # ones columns in vaug (per head col 64), then bias broadcasts (Pool)
    vhe = vaug[:, :, :].rearrange("p t (h e) -> p t h e", e=VW)
    nc.gpsimd.memset(vhe[:, :, :, HD : HD + 1], 1.0)
    nc.gpsimd.partition_broadcast(out_ap=bv_b, in_ap=brow[0:1, 0, :])
    nc.gpsimd.partition_broadcast(out_ap=bp_b, in_ap=brow[0:1, 1, :])

    # PSUM pools -----------------------------------------------------------
    pbig = ctx.enter_context(tc.tile_pool(name="pbig", bufs=2, space="PSUM"))
    paux = ctx.enter_context(tc.tile_pool(name="paux", bufs=1, space="PSUM"))
    ppvy = ctx.enter_context(tc.tile_pool(name="ppvy", bufs=2, space="PSUM"))

    # ---- building blocks -------------------------------------------------
    def qk_tile(m, n):
        ps = paux.tile([P, TQB], F32, name=f"qkps{m}_{n}", tag="aux")
        for t in range(KC // 2):
            nc.tensor.matmul(
                ps[:, :],
                w8_sb[:, t, :, m * P : (m + 1) * P],
                xT8[:, t, :, n * TQB : (n + 1) * TQB],
                start=(t == 0),
                stop=(t == KC // 2 - 1),
                perf_mode=mybir.MatmulPerfMode.DoubleRow,
            )
        # bias-add + PSUM->SBUF move on DVE (ACT is the exp bottleneck)
        nc.vector.tensor_tensor(
            out=qkT[:, m, n * TQB : (n + 1) * TQB],
            in0=ps[:, :],
            in1=battn_pm[:, m : m + 1].to_broadcast([P, TQB]),
            op=ALU.add,
        )

    def v_tile(tt, n):
        for _, cl in v_items(tt, n):
            cl()

    def v_items(tt, n):
        """v projection half-tile as 3 small work items (2 k-chunks each)."""
        nsz = min(TQB, C - n * TQB)  # 512, 256
        state = {}

        def chunk(j):
            if "ps" not in state:
                state["ps"] = paux.tile(
                    [P, TQB], F32, name=f"vps{tt}_{n}", tag="aux"
                )
            ps = state["ps"]
            for k in (2 * j, 2 * j + 1):
                nc.tensor.matmul(
                    ps[:, :nsz],
                    xT[:, k, tt * P : (tt + 1) * P],
                    wv_sb[:, k, n * TQB : n * TQB + nsz],
                    start=(k == 0),
                    stop=(k == KC - 1),
                )
            if j == 2:
                nh0 = n * TQB // HD
                nh = nsz // HD
                nc.vector.tensor_tensor(
                    out=vhe[:, tt, nh0 : nh0 + nh, 0:HD],
                    in0=ps[:, :nsz].rearrange("p (h e) -> p h e", e=HD),
                    in1=bv_b[:, n * TQB : n * TQB + nsz].rearrange(
                        "p (h e) -> p h e", e=HD
                    ),
                    op=ALU.add,
                )

        est = (430, 215)[n]
        return [(est, lambda j=j: chunk(j)) for j in range(3)]

    def out_tile(m, pool=None, tag="aux"):
        for _, cl in out_items(m, pool, tag):
            cl()

    def out_items(m, pool=None, tag="aux"):
        """out-projection tile as 6 small work items (2 k-chunks per n)."""
        state = {}

        def chunk(n, j):
            if "ps" not in state:
                state["ps"] = (pool or paux).tile(
                    [P, NB, TQB], F32, name=f"ops{m}", tag=tag
                )
                state["ot"] = otpool.tile([P, C], F32, name=f"ot{m}", tag="ot")
            ps, ot = state["ps"], state["ot"]
            nsz = min(TQB, C - n * TQB)
            for k in (2 * j, 2 * j + 1):
                nc.tensor.matmul(
                    ps[:, n, :nsz],
                    yT[:, k, m * P : (m + 1) * P],
                    wp_sb[:, k, n * TQB : n * TQB + nsz],
                    start=(k == 0),
                    stop=(k == KC - 1),
                )
            if j == 2:
                nc.vector.tensor_tensor(
                    out=ot[:, n * TQB : n * TQB + nsz],
                    in0=ps[:, n, :nsz],
                    in1=bp_b[:, n * TQB : n * TQB + nsz],
                    op=ALU.add,
                )
                nc.sync.dma_start(
                    out=out_d.rearrange("(t p) c -> p t c", p=P)[
                        :, m : m + 1, n * TQB : n * TQB + nsz
                    ],
                    in_=ot[:, None, n * TQB : n * TQB + nsz],
                )

        return [
            ((430, 215)[n], lambda n=n, j=j: chunk(n, j))
            for n in range(NB)
            for j in range(3)
        ]

    def attn_ST(hp, b, workq):
        """ST + exp + mask for one head pair and tq block; returns the ut
        tiles for attn_PV. Between ST tiles, drains work items from `workq`
        up to the PE slack under each exp's latency."""
        ntk = 4 * (b + 1)
        uts = []
        for tk in range(ntk):
            off = max(0, tk * P - b * TQB)
            nn = TQB - off
            diag = tk * P >= b * TQB
            pst = pbig.tile([P, 2, TQB], F32, name="pst", tag="big")
            for h in range(2):
                lo, hi = 64 * h, 64 * h + 64
                nc.tensor.matmul(
                    pst[:, h, off:TQB],
                    qkT[lo:hi, 6 + hp, tk * P : (tk + 1) * P],
                    qkT[lo:hi, hp, b * TQB + off : (b + 1) * TQB],
                    start=True,
                    stop=True,
                )
            ut = upool.tile([P, 2, TQB], BF16, name="ut")
            nc.scalar.activation(
                out=ut[:, :, off:TQB],
                in_=pst[:, :, off:TQB],
                func=FT.Exp,
                scale=0.125,
            )
            if diag:
                # zero the strictly-upper triangle of the diagonal block
                # (keep where tq_col - tk_row >= 0), on the Pool engine
                nc.gpsimd.affine_select(
                    out=ut[:, :, off : off + P],
                    in_=ut[:, :, off : off + P],
                    compare_op=ALU.is_ge,
                    fill=0.0,
                    base=0,
                    pattern=[[0, 2], [1, P]],
                    channel_multiplier=-1,
                )
            uts.append(ut)
            if workq:
                workq.pop(0)()
        for cl in workq:
            cl()
        del workq[:]
        return uts

    def attn_PV_items(hp, b, uts, dma_ytr=False):
        """PV accumulation + normalization for one unit as a list of small
        closures, so they can be woven between the next unit's ST tiles."""
        state = {}

        def group(j):
            if "pvy" not in state:
                state["pvy"] = [
                    ppvy.tile(
                        [P, 2, 2, VW], F32, name=f"pvy{t}_{hp}_{b}", tag="pvy"
                    )
                    for t in range(2)
                ]
            pvy = state["pvy"]
            # one PSUM accumulation group per (j, h): all tk's consecutively
            # (PSUM pending-zero is bank-granular, groups in a bank must not
            # interleave)
            last = 4 * b + j
            for h in range(2):
                for tk in range(last + 1):
                    nc.tensor.matmul(
                        pvy[j // 2][:, j % 2, h, 0:VW],
                        uts[tk][:, h, j * P : (j + 1) * P],
                        vaug[:, tk, (2 * hp + h) * VW : (2 * hp + h + 1) * VW],
                        start=(tk == 0),
                        stop=(tk == last),
                    )

        def norm():
            pvy = state["pvy"]
            # per-partition reciprocal of the row-sum column, then one
            # broadcast multiply per psum tile
            rc = rcpool.tile([P, NJ, 2, 1], F32, name="rc")
            ysb = ypool.tile([P, NJ, 2, HD], BF16, name="ysb")
            with tc.high_priority():
                with nc.allow_low_precision(reason="softmax normalization"):
                    for t in range(2):
                        nc.vector.reciprocal(
                            out=rc[:, 2 * t : 2 * t + 2, :, :],
                            in_=pvy[t][:, :, :, HD : HD + 1],
                        )
                for t in range(2):
                    nc.vector.tensor_tensor(
                        out=ysb[:, 2 * t : 2 * t + 2, :, :],
                        in0=pvy[t][:, :, :, 0:HD],
                        in1=rc[:, 2 * t : 2 * t + 2, :, :].to_broadcast(
                            [P, 2, 2, HD]
                        ),
                        op=ALU.mult,
                    )
                # transpose y [tq, (h hd)] -> yT [(h hd), tq] per tq chunk
                if dma_ytr:
                    # off the PE/DVE path: SBUF->SBUF XBAR transpose (the
                    # DMA path is idle mid-stream; not used for the last
                    # units whose yT is on the tail critical path)
                    for j in range(NJ):
                        nc.sync.dma_start_transpose(
                            out=yT[
                                :, hp, b * TQB + j * P : b * TQB + (j + 1) * P
                            ],
                            in_=ysb[:, j, :, :],
                        )
                else:
                    ytr = ppvy.tile([P, NJ, P], BF16, name="ytr", tag="pvy")
                    for j in range(NJ):
                        nc.tensor.transpose(
                            ytr[:, j, :], ysb[:, j, :, :], ident[:]
                        )
                    nc.vector.tensor_copy(
                        out=yT[:, hp, b * TQB : (b + 1) * TQB].rearrange(
                            "p (j f) -> p j f", j=NJ
                        ),
                        in_=ytr[:, :, :],
                    )

        return [
            ((4 * b + j + 1) * 2 * 27 + 30, lambda j=j: group(j))
            for j in range(NJ)
        ] + [(350, norm)]

    # ---- emission schedule ----------------------------------------------
    # One list of (hp, b) attention units in processing order; ST blocks are
    # software-pipelined one unit ahead of PV blocks so the PE stream never
    # waits on the exp (ACT) chain. Projection halves and out-proj tiles
    # ride as PE filler inside the ST tk loops.
    qk_tile(0, 0)
    qk_tile(6, 0)
    v_tile(0, 0)
    v_tile(1, 0)
    v_tile(2, 0)
    v_tile(3, 0)

    # unit order front-loads the exp-heavy b=1 blocks so the ACT exp stream
    # has no holes; (5,0) stays ahead of the last two b=1 units so the b=0
    # output tiles get a head start before the tail
    UNITS = [
        (0, 0), (1, 0), (0, 1), (1, 1), (2, 0), (2, 1),
        (3, 0), (3, 1), (4, 0), (5, 0), (4, 1), (5, 1),
    ]
    FILL = {
        (0, 0): [(qk_tile, 1, 0), (qk_tile, 7, 0), (qk_tile, 0, 1),
                 (qk_tile, 6, 1)],
        (1, 0): [(qk_tile, 1, 1), (qk_tile, 7, 1), (v_tile, 0, 0),
                 (v_tile, 1, 0), (v_tile, 2, 0), (v_tile, 3, 0)],
        (0, 1): [(qk_tile, 2, 0), (qk_tile, 8, 0), (v_tile, 4, 0),
                 (v_tile, 5, 0)],
        (1, 1): [(qk_tile, 2, 1), (qk_tile, 8, 1), (v_tile, 6, 0),
                 (v_tile, 7, 0)],
        (2, 0): [(qk_tile, 3, 0), (qk_tile, 9, 0), (v_tile, 0, 1)],
        (2, 1): [(qk_tile, 3, 1), (qk_tile, 9, 1), (v_tile, 1, 1)],
        (3, 0): [(qk_tile, 4, 0), (qk_tile, 10, 0), (v_tile, 2, 1)],
        (3, 1): [(qk_tile, 5, 0), (qk_tile, 11, 0), (v_tile, 3, 1)],
        (4, 0): [(qk_tile, 4, 1), (qk_tile, 10, 1), (v_tile, 4, 1)],
        (5, 0): [(qk_tile, 5, 1), (qk_tile, 11, 1), (v_tile, 5, 1)],
        (4, 1): [(v_tile, 6, 1), (v_tile, 7, 1)],
        (5, 1): [(out_tile, 0), (out_tile, 1), (out_tile, 2),
                 (out_tile, 3)],
    }

    def _weave(a, b_):
        out = []
        while a or b_:
            if a:
                out.append(a.pop(0))
            if b_:
                out.append(b_.pop(0))
        return out

    qk_tile(0, 0)
    qk_tile(6, 0)
    # lag-2 PV weaving: unit u's PV work is woven during unit u+2, so the
    # early ST blocks (and their exps) are not held back by the v-tile
    # backlog; the last two units drain double so the tail stays one PV deep
    from collections import deque

    pend = deque()
    for ui, unit in enumerate(UNITS):
        hp, b = unit
        drain = []
        while pend and (len(pend) >= 2 or ui >= len(UNITS) - 2):
            drain.extend(cl for _, cl in pend.popleft())
        fillers = [(lambda f=u[0], args=u[1:]: f(*args)) for u in FILL[unit]]
        work = _weave(drain, fillers)
        uts = attn_ST(hp, b, work)
        pend.append(attn_PV_items(hp, b, uts, dma_ytr=ui < len(UNITS) - 2))
    for lst in pend:
        for _, cl in lst:
            cl()
    for m in range(4, KT):
        if m % 2 == 1:
            out_tile(m, pool=pbig, tag="big")
        else:
            out_tile(m)

_prog_cache = {}


def _get_program():
    if "nc" not in _prog_cache:
        _prog_cache["nc"] = build_program()
    return _prog_cache["nc"]


def kernel(x, w_attn, b_attn, w_proj, b_proj, _trace=False):
    nc = _get_program()
    bf = ml_dtypes.bfloat16
    xb = np.ascontiguousarray(
        np.asarray(x, dtype=np.float32).astype(bf).transpose(0, 2, 1)
    )
    wab = np.ascontiguousarray(np.asarray(w_attn, dtype=np.float32).astype(bf))
    w8 = np.ascontiguousarray(
        np.asarray(w_attn[:, : 2 * C], dtype=np.float32).astype(
            ml_dtypes.float8_e4m3
        )
    )
    wpb = np.ascontiguousarray(np.asarray(w_proj, dtype=np.float32).astype(bf))
    b_attn = np.ascontiguousarray(np.asarray(b_attn, dtype=np.float32))
    b_proj = np.ascontiguousarray(np.asarray(b_proj, dtype=np.float32))
    in_maps = [
        {
            "xbT": xb[b],
            "wab": wab,
            "w8": w8,
            "b_attn": b_attn,
            "wpb": wpb,
            "b_proj": b_proj,
        }
        for b in range(B)
    ]
    res = run_bass_kernel_spmd(nc, in_maps, list(range(B)), trace=_trace)
    out = np.stack([res.results[i]["out"] for i in range(B)], axis=0)
    if _trace:
        kernel.last_results = res
    return out


# revision 45
# speedup vs baseline: 1.0177x; 1.0167x over previous
"""Causal self-attention (B=8, T=1024, C=768, NH=12) on 8 TRN2 NeuronCores.

Sharding: pure data parallel - one batch element per core, no collectives.

Host side: x is pre-transposed to [C, T] and shipped twice - as bf16 (for
the v projection) and as fp8e4m3 (for the qk projection); w_attn/w_proj are
cast to bf16 and the q/k half of w_attn additionally to fp8e4m3 (w8).
Biases stay fp32.

Per-core kernel (Bass/Tile):
  - The fp8 qk inputs (w8 + x8, 1.9 MB) load first so the exp stream starts
    ~8.5us in; bf16 x and wv stream in behind them for the v projection.
  - qk projection in fp8e4m3 with MatmulPerfMode.DoubleRow: 3 accumulation
    steps of K=256 ([128, 2, *] operand pairs), 4x fewer PE cycles than the
    bf16 equivalent. Bias-add + PSUM->SBUF move on DVE (per-partition bias
    battn_pm). v projection and output projection stay bf16 (fp8 there fails
    the 2e-2 accuracy budget; qk errors are damped through exp).
  - Attention per (head pair, 512-wide tq block) "unit": ST = kT.T @ qT ->
    PSUM [tk=128, tq] (2 heads in PE row groups 0-63/64-127), exp on ACT
    into SBUF `ut` (the engine-time floor: ~60us of exp), diagonal-block
    causal mask via gpsimd affine_select on the idle Pool engine.
  - PV is oriented [tq, hd]: per 128-wide tq chunk j and head,
    y[tq, 65] += U[tk, tq-chunk].T @ v_aug[tk, head] accumulated over tk as
    one PSUM group per (j, head) (PSUM pending-zero is bank-granular, so
    groups in a bank must not interleave). Column 64 (ones in v_aug) yields
    the softmax row-sums, so normalization is per-partition: reciprocal +
    one broadcast multiply per psum tile on DVE - no PE broadcast matmuls.
  - y[tq, (h,hd)] tiles are transposed back to yT [c-chunk, tq] for the
    output projection via SBUF->SBUF DMA XBAR transposes (the DMA path is
    idle mid-stream); only the last two units - whose yT sits on the tail
    critical path - use the lower-latency PE-transpose + DVE-copy path.
    Output projection: stationary yT, moving w_proj, bias on DVE, DMA out.
  - PSUM (8 banks): "big" = ST psum ring (2 slots x 2 banks, nothing else -
    sharing it halves the ST/exp double-buffering), "aux" (1 slot x 2 banks)
    for qk/v/out projection psums, "pvy" (2 slots x 1 bank) for the PV
    accumulators and the y-transpose psum.
  - Schedule: emission order defines both scheduler priority AND the
    dataflow (a consumer emitted before its producer is a race, not a
    reorderable dependency - CoreSim's initialized-memory check catches
    this). Attention units alternate b=0 (exp-light) and b=1 (exp-heavy)
    blocks so the ACT exp stream has no holes, and PV work is woven with a
    LAG OF TWO units (PV of unit u between unit u+2's ST tiles, draining
    double at the end) so the early ST blocks are not held back by the
    DMA-gated v-tile backlog. qk/v projection halves and out-proj tiles
    ride as fillers in the remaining ST slots.

TimelineSim: 101.5us vs 168.3us for the previous kernel (1.66x). HW-verified
rel err 9.7e-3 (fp8 qk path dominates; bf16-only baseline was 3.1e-3).
"""

import numpy as np
import ml_dtypes

import concourse.bass as bass
import concourse.bacc as bacc
import concourse.tile as tile
from concourse import mybir
from concourse.bass_utils import run_bass_kernel_spmd

B, T, C = 8, 1024, 768
NH, HD = 12, 64
P = 128
KC = C // P          # 6 k-tiles over C
KT = T // P          # 8 tiles over T
NQK = 2 * C // P     # 12 m-tiles for q+k
NHP = NH // 2        # 6 head pairs
TQB = 512            # tq block (one PSUM bank of fp32)
NB = T // TQB        # 2 tq blocks
NJ = TQB // P        # 4 tq chunks per block
VW = HD + 1          # 65: v columns + ones column per head

F32 = mybir.dt.float32
BF16 = mybir.dt.bfloat16
FP8 = mybir.dt.float8e4
FT = mybir.ActivationFunctionType
ALU = mybir.AluOpType


def build_program():
    nc = bacc.Bacc("TRN2", target_bir_lowering=False, debug=False)
    xb_d = nc.dram_tensor("xbT", [C, T], BF16, kind="ExternalInput").ap()
    wab_d = nc.dram_tensor("wab", [C, 3 * C], BF16, kind="ExternalInput").ap()
    w8_d = nc.dram_tensor("w8", [C, 2 * C], FP8, kind="ExternalInput").ap()
    ba_d = nc.dram_tensor("b_attn", [3 * C], F32, kind="ExternalInput").ap()
    wpb_d = nc.dram_tensor("wpb", [C, C], BF16, kind="ExternalInput").ap()
    bp_d = nc.dram_tensor("b_proj", [C], F32, kind="ExternalInput").ap()
    out_d = nc.dram_tensor("out", [T, C], F32, kind="ExternalOutput").ap()

    from contextlib import ExitStack

    with tile.TileContext(nc) as tc:
        with ExitStack() as ctx:
            _body(ctx, tc, xb_d, wab_d, w8_d, ba_d, wpb_d, bp_d, out_d)
    nc.compile()
    return nc


def _body(ctx, tc, xb_d, wab_d, w8_d, ba_d, wpb_d, bp_d, out_d):
    nc = tc.nc

    const = ctx.enter_context(tc.tile_pool(name="const", bufs=1))
    persist = ctx.enter_context(tc.tile_pool(name="persist", bufs=1))
    upool = ctx.enter_context(tc.tile_pool(name="upool", bufs=24))
    ypool = ctx.enter_context(tc.tile_pool(name="ypool", bufs=4))
    rcpool = ctx.enter_context(tc.tile_pool(name="rcpool", bufs=2))
    otpool = ctx.enter_context(tc.tile_pool(name="otpool", bufs=4))

    # constants ------------------------------------------------------------
    ident = const.tile([P, P], BF16)
    nc.gpsimd.memset(ident, 0.0)
    nc.gpsimd.affine_select(
        out=ident, in_=ident, compare_op=ALU.not_equal,
        fill=1.0, base=0, pattern=[[-1, P]], channel_multiplier=1,
    )
    # b_attn for q/k as per-partition scalars: [p, m] with b[128m + p]
    battn_pm = const.tile([P, NQK], F32)
    # biases broadcast along partitions on-chip (a stride-0 partition DMA is
    # modeled/executed as a full 128x transfer): DMA one row, gpsimd bcast
    brow = const.tile([P, 2, C], F32)
    bv_b = const.tile([P, C], F32)
    bp_b = const.tile([P, C], F32)

    # persistent SBUF tensors ---------------------------------------------
    xT = persist.tile([P, KC, T], BF16)          # [128, 6, 1024] 12 KB/par
    xT8 = persist.tile([P, KC // 2, 2, T], FP8)  # fp8 copy for qk DoubleRow
    w8_sb = persist.tile([P, KC // 2, 2, 2 * C], FP8)  # [128, 3, 2, 1536]
    wv_sb = persist.tile([P, KC, C], BF16)       # 9 KB
    wp_sb = persist.tile([P, KC, C], BF16)       # 9 KB
    qkT = persist.tile([P, NQK, T], BF16)        # [128, 12, 1024] 24 KB
    vaug = persist.tile([P, KT, NH * VW], BF16)  # [128, 8, 780] 12.2 KB
    yT = persist.tile([P, NHP, T], BF16)         # [128, 6, 1024] 12 KB

    # input DMAs in priority order; x arrives host-transposed [C, T] so xT
    # chunks are plain contiguous loads; the qk weight half loads as 6
    # contiguous row-chunk DMAs (all 12 m-tiles at once)
    for k in range(KC):
        if k % 2 == 0:
            t = k // 2
            nc.sync.dma_start(
                out=w8_sb[:, t, :, :],
                in_=w8_d[t * 2 * P : (t + 1) * 2 * P, :].rearrange(
                    "(i p) m -> p i m", p=P
                ),
            )
        nc.sync.dma_start(
            out=xT[:, k, :], in_=xb_d[k * P : (k + 1) * P, :]
        )
        # fp8 copy for the qk projection (channel order is unchanged:
        # free index (t, i) == chunk 2t+i); alternate Pool/DVE
        eng = nc.gpsimd if k % 2 == 0 else nc.vector
        eng.tensor_copy(out=xT8[:, k // 2, k % 2, :], in_=xT[:, k, :])
    nc.sync.dma_start(out=brow[0:1, 0, :], in_=ba_d[None, 2 * C : 3 * C])
    nc.sync.dma_start(out=brow[0:1, 1, :], in_=bp_d[None, :])
    nc.sync.dma_start(
        out=battn_pm, in_=ba_d[0 : 2 * C].rearrange("(m p) -> p m", p=P)
    )
    for k in range(KC):
        nc.sync.dma_start(
            out=wv_sb[:, k, :], in_=wab_d[k * P : (k + 1) * P, 2 * C : 3 * C]
        )
    for k in range(KC):
        nc.sync.dma_start(out=wp_sb[:, k, :], in_=wpb_d[k * P : (k + 1) * P, :])

    # ones columns in vaug (per head col 64), then bias broadcasts (Pool)
    vhe = vaug[:, :, :].rearrange("p t (h e) -> p t h e", e=VW)
    nc.gpsimd.memset(vhe[:, :, :, HD : HD + 1], 1.0)
    nc.gpsimd.partition_broadcast(out_ap=bv_b, in_ap=brow[0:1, 0, :])
    nc.gpsimd.partition_broadcast(out_ap=bp_b, in_ap=brow[0:1, 1, :])

    # PSUM pools -----------------------------------------------------------
    pbig = ctx.enter_context(tc.tile_pool(name="pbig", bufs=2, space="PSUM"))
    paux = ctx.enter_context(tc.tile_pool(name="paux", bufs=1, space="PSUM"))
    ppvy = ctx.enter_context(tc.tile_pool(name="ppvy", bufs=2, space="PSUM"))

    # ---- building blocks -------------------------------------------------
    def qk_tile(m, n):
        ps = paux.tile([P, TQB], F32, name=f"qkps{m}_{n}", tag="aux")
        for t in range(KC // 2):
            nc.tensor.matmul(
                ps[:, :],
                w8_sb[:, t, :, m * P : (m + 1) * P],
                xT8[:, t, :, n * TQB : (n + 1) * TQB],
                start=(t == 0),
                stop=(t == KC // 2 - 1),
                perf_mode=mybir.MatmulPerfMode.DoubleRow,
            )
        # bias-add + PSUM->SBUF move on DVE (ACT is the exp bottleneck)
        nc.vector.tensor_tensor(
            out=qkT[:, m, n * TQB : (n + 1) * TQB],
            in0=ps[:, :],
            in1=battn_pm[:, m : m + 1].to_broadcast([P, TQB]),
            op=ALU.add,
        )

    def v_tile(tt, n):
        for _, cl in v_items(tt, n):
            cl()

    def v_items(tt, n):
        """v projection half-tile as 3 small work items (2 k-chunks each)."""
        nsz = min(TQB, C - n * TQB)  # 512, 256
        state = {}

        def chunk(j):
            if "ps" not in state:
                state["ps"] = paux.tile(
                    [P, TQB], F32, name=f"vps{tt}_{n}", tag="aux"
                )
            ps = state["ps"]
            for k in (2 * j, 2 * j + 1):
                nc.tensor.matmul(
                    ps[:, :nsz],
                    xT[:, k, tt * P : (tt + 1) * P],
                    wv_sb[:, k, n * TQB : n * TQB + nsz],
                    start=(k == 0),
                    stop=(k == KC - 1),
                )
            if j == 2:
                nh0 = n * TQB // HD
                nh = nsz // HD
                nc.vector.tensor_tensor(
                    out=vhe[:, tt, nh0 : nh0 + nh, 0:HD],
                    in0=ps[:, :nsz].rearrange("p (h e) -> p h e", e=HD),
                    in1=bv_b[:, n * TQB : n * TQB + nsz].rearrange(
                        "p (h e) -> p h e", e=HD
                    ),
                    op=ALU.add,
                )

        est = (430, 215)[n]
        return [(est, lambda j=j: chunk(j)) for j in range(3)]

    def out_tile(m, pool=None, tag="aux"):
        for _, cl in out_items(m, pool, tag):
            cl()

    def out_items(m, pool=None, tag="aux"):
        """out-projection tile as 6 small work items (2 k-chunks per n)."""
        state = {}

        def chunk(n, j):
            if "ps" not in state:
                state["ps"] = (pool or paux).tile(
                    [P, NB, TQB], F32, name=f"ops{m}", tag=tag
                )
                state["ot"] = otpool.tile([P, C], F32, name=f"ot{m}", tag="ot")
            ps, ot = state["ps"], state["ot"]
            nsz = min(TQB, C - n * TQB)
            for k in (2 * j, 2 * j + 1):
                nc.tensor.matmul(
                    ps[:, n, :nsz],
                    yT[:, k, m * P : (m + 1) * P],
                    wp_sb[:, k, n * TQB : n * TQB + nsz],
                    start=(k == 0),
                    stop=(k == KC - 1),
                )
            if j == 2:
                nc.vector.tensor_tensor(
                    out=ot[:, n * TQB : n * TQB + nsz],
                    in0=ps[:, n, :nsz],
                    in1=bp_b[:, n * TQB : n * TQB + nsz],
                    op=ALU.add,
                )
                nc.sync.dma_start(
                    out=out_d.rearrange("(t p) c -> p t c", p=P)[
                        :, m : m + 1, n * TQB : n * TQB + nsz
                    ],
                    in_=ot[:, None, n * TQB : n * TQB + nsz],
                )

        return [
            ((430, 215)[n], lambda n=n, j=j: chunk(n, j))
            for n in range(NB)
            for j in range(3)
        ]

    def attn_ST(hp, b, workq):
        """ST + exp + mask for one head pair and tq block; returns the ut
        tiles for attn_PV. Between ST tiles, drains work items from `workq`
        up to the PE slack under each exp's latency."""
        ntk = 4 * (b + 1)
        uts = []
        for tk in range(ntk):
            off = max(0, tk * P - b * TQB)
            nn = TQB - off
            diag = tk * P >= b * TQB
            pst = pbig.tile([P, 2, TQB], F32, name="pst", tag="big")
            for h in range(2):
                lo, hi = 64 * h, 64 * h + 64
                nc.tensor.matmul(
                    pst[:, h, off:TQB],
                    qkT[lo:hi, 6 + hp, tk * P : (tk + 1) * P],
                    qkT[lo:hi, hp, b * TQB + off : (b + 1) * TQB],
                    start=True,
                    stop=True,
                )
            ut = upool.tile([P, 2, TQB], BF16, name="ut")
            nc.scalar.activation(
                out=ut[:, :, off:TQB],
                in_=pst[:, :, off:TQB],
                func=FT.Exp,
                scale=0.125,
            )
            if diag:
                # zero the strictly-upper triangle of the diagonal block
                # (keep where tq_col - tk_row >= 0), on the Pool engine
                nc.gpsimd.affine_select(
                    out=ut[:, :, off : off + P],
                    in_=ut[:, :, off : off + P],
                    compare_op=ALU.is_ge,
                    fill=0.0,
                    base=0,
                    pattern=[[0, 2], [1, P]],
                    channel_multiplier=-1,
                )
            uts.append(ut)
            if workq:
                workq.pop(0)()
        for cl in workq:
            cl()
        del workq[:]
        return uts

    def attn_PV_items(hp, b, uts, dma_ytr=False):
        """PV accumulation + normalization for one unit as a list of small
        closures, so they can be woven between the next unit's ST tiles."""
        state = {}

        def group(j):
            if "pvy" not in state:
                state["pvy"] = [
                    ppvy.tile(
                        [P, 2, 2, VW], F32, name=f"pvy{t}_{hp}_{b}", tag="pvy"
                    )
                    for t in range(2)
                ]
            pvy = state["pvy"]
            # one PSUM accumulation group per (j, h): all tk's consecutively
            # (PSUM pending-zero is bank-granular, groups in a bank must not
            # interleave)
            last = 4 * b + j
            for h in range(2):
                for tk in range(last + 1):
                    nc.tensor.matmul(
                        pvy[j // 2][:, j % 2, h, 0:VW],
                        uts[tk][:, h, j * P : (j + 1) * P],
                        vaug[:, tk, (2 * hp + h) * VW : (2 * hp + h + 1) * VW],
                        start=(tk == 0),
                        stop=(tk == last),
                    )

        def norm():
            pvy = state["pvy"]
            # per-partition reciprocal of the row-sum column, then one
            # broadcast multiply per psum tile
            rc = rcpool.tile([P, NJ, 2, 1], F32, name="rc")
            ysb = ypool.tile([P, NJ, 2, HD], BF16, name="ysb")
            with tc.high_priority():
                with nc.allow_low_precision(reason="softmax normalization"):
                    for t in range(2):
                        nc.vector.reciprocal(
                            out=rc[:, 2 * t : 2 * t + 2, :, :],
                            in_=pvy[t][:, :, :, HD : HD + 1],
                        )
                for t in range(2):
                    nc.vector.tensor_tensor(
                        out=ysb[:, 2 * t : 2 * t + 2, :, :],
                        in0=pvy[t][:, :, :, 0:HD],
                        in1=rc[:, 2 * t : 2 * t + 2, :, :].to_broadcast(
                            [P, 2, 2, HD]
                        ),
                        op=ALU.mult,
                    )
                # transpose y [tq, (h hd)] -> yT [(h hd), tq] per tq chunk
                if dma_ytr:
                    # off the PE/DVE path: SBUF->SBUF XBAR transpose (the
                    # DMA path is idle mid-stream; not used for the last
                    # units whose yT is on the tail critical path)
                    for j in range(NJ):
                        nc.sync.dma_start_transpose(
                            out=yT[
                                :, hp, b * TQB + j * P : b * TQB + (j + 1) * P
                            ],
                            in_=ysb[:, j, :, :],
                        )
                else:
                    ytr = ppvy.tile([P, NJ, P], BF16, name="ytr", tag="pvy")
                    for j in range(NJ):
                        nc.tensor.transpose(
                            ytr[:, j, :], ysb[:, j, :, :], ident[:]
                        )
                    nc.vector.tensor_copy(
                        out=yT[:, hp, b * TQB : (b + 1) * TQB].rearrange(
                            "p (j f) -> p j f", j=NJ
                        ),
                        in_=ytr[:, :, :],
                    )

        return [
            ((4 * b + j + 1) * 2 * 27 + 30, lambda j=j: group(j))
            for j in range(NJ)
        ] + [(350, norm)]

    # ---- emission schedule ----------------------------------------------
    # One list of (hp, b) attention units in processing order; ST blocks are
    # software-pipelined one unit ahead of PV blocks so the PE stream never
    # waits on the exp (ACT) chain. Projection halves and out-proj tiles
    # ride as PE filler inside the ST tk loops.
    qk_tile(0, 0)
    qk_tile(6, 0)
    v_tile(0, 0)
    v_tile(1, 0)
    v_tile(2, 0)
    v_tile(3, 0)

    # unit order front-loads the exp-heavy b=1 blocks so the ACT exp stream
    # has no holes; (5,0) stays ahead of the last two b=1 units so the b=0
    # output tiles get a head start before the tail
    UNITS = [
        (0, 0), (1, 0), (0, 1), (1, 1), (2, 0), (2, 1),
        (3, 0), (3, 1), (4, 0), (5, 0), (4, 1), (5, 1),
    ]
    FILL = {
        (0, 0): [(qk_tile, 1, 0), (qk_tile, 7, 0), (qk_tile, 0, 1),
                 (qk_tile, 6, 1)],
        (1, 0): [(qk_tile, 1, 1), (qk_tile, 7, 1), (v_tile, 0, 0),
                 (v_tile, 1, 0), (v_tile, 2, 0), (v_tile, 3, 0)],
        (0, 1): [(qk_tile, 2, 0), (qk_tile, 8, 0), (v_tile, 4, 0),
                 (v_tile, 5, 0)],
        (1, 1): [(qk_tile, 2, 1), (qk_tile, 8, 1), (v_tile, 6, 0),
                 (v_tile, 7, 0)],
        (2, 0): [(qk_tile, 3, 0), (qk_tile, 9, 0), (v_tile, 0, 1)],
        (2, 1): [(qk_tile, 3, 1), (qk_tile, 9, 1), (v_tile, 1, 1)],
        (3, 0): [(qk_tile, 4, 0), (qk_tile, 10, 0), (v_tile, 2, 1)],
        (3, 1): [(qk_tile, 5, 0), (qk_tile, 11, 0), (v_tile, 3, 1)],
        (4, 0): [(qk_tile, 4, 1), (qk_tile, 10, 1), (v_tile, 4, 1)],
        (5, 0): [(qk_tile, 5, 1), (qk_tile, 11, 1), (v_tile, 5, 1)],
        (4, 1): [(v_tile, 6, 1), (v_tile, 7, 1)],
        (5, 1): [(out_tile, 0), (out_tile, 1), (out_tile, 2),
                 (out_tile, 3)],
    }

    def _weave(a, b_):
        out = []
        while a or b_:
            if a:
                out.append(a.pop(0))
            if b_:
                out.append(b_.pop(0))
        return out

    qk_tile(0, 0)
    qk_tile(6, 0)
    # lag-2 PV weaving: unit u's PV work is woven during unit u+2, so the
    # early ST blocks (and their exps) are not held back by the v-tile
    # backlog; the last two units drain double so the tail stays one PV deep
    from collections import deque

    pend = deque()
    for ui, unit in enumerate(UNITS):
        hp, b = unit
        drain = []
        while pend and (len(pend) >= 2 or ui >= len(UNITS) - 2):
            drain.extend(cl for _, cl in pend.popleft())
        fillers = [(lambda f=u[0], args=u[1:]: f(*args)) for u in FILL[unit]]
        work = _weave(drain, fillers)
        uts = attn_ST(hp, b, work)
        pend.append(attn_PV_items(hp, b, uts, dma_ytr=ui < len(UNITS) - 2))
    for lst in pend:
        for _, cl in lst:
            cl()
    for m in range(4, KT):
        if m % 2 == 1:
            out_tile(m, pool=pbig, tag="big")
        else:
            out_tile(m)

_prog_cache = {}


def _get_program():
    if "nc" not in _prog_cache:
        _prog_cache["nc"] = build_program()
    return _prog_cache["nc"]


def kernel(x, w_attn, b_attn, w_proj, b_proj, _trace=False):
    nc = _get_program()
    bf = ml_dtypes.bfloat16
    xb = np.ascontiguousarray(
        np.asarray(x, dtype=np.float32).astype(bf).transpose(0, 2, 1)
    )
    wab = np.ascontiguousarray(np.asarray(w_attn, dtype=np.float32).astype(bf))
    w8 = np.ascontiguousarray(
        np.asarray(w_attn[:, : 2 * C], dtype=np.float32).astype(
            ml_dtypes.float8_e4m3
        )
    )
    wpb = np.ascontiguousarray(np.asarray(w_proj, dtype=np.float32).astype(bf))
    b_attn = np.ascontiguousarray(np.asarray(b_attn, dtype=np.float32))
    b_proj = np.ascontiguousarray(np.asarray(b_proj, dtype=np.float32))
    in_maps = [
        {
            "xbT": xb[b],
            "wab": wab,
            "w8": w8,
            "b_attn": b_attn,
            "wpb": wpb,
            "b_proj": b_proj,
        }
        for b in range(B)
    ]
    res = run_bass_kernel_spmd(nc, in_maps, list(range(B)), trace=_trace)
    out = np.stack([res.results[i]["out"] for i in range(B)], axis=0)
    if _trace:
        kernel.last_results = res
    return out
